# revision 24
# baseline (speedup 1.0000x reference)
"""BEiT-style attention (B=64, N=197, C=768, H=12, rel-pos bias) on 8 TRN2 cores.

Data-parallel over batch: 8 batch items per core, no collectives.

Key layout / engine choices:
  - The qk projection runs in fp8e4 DoubleRow perf mode (two contraction
    rows per partition, 2x PE throughput), weights pre-scaled by 64 on the
    host; the 1/64 (and the 1/sqrt(hd) attention scale for q) folds back in
    via the ACT scale operand during PSUM evacuation. The v projection and
    everything downstream stay bf16: softmax weights are near-uniform here,
    so v/proj quantization error does not average away, but q/k error only
    perturbs softmax weights (~1.6e-2 total, inside the 2e-2 gate).
  - qkv computed transposed: qkT [j, m]; per-head qT/kT [hd, n] slices are
    direct row-slices. v computed un-transposed [m, d] per batch.
  - Attention runs in the S^T layout: S^T[m, n] = k q^T (keys on
    partitions). The rel-pos bias is applied multiplicatively after exp
    (exp(S+rpb) = exp(S)*exp(rpb)) by one DVE multiply against a
    host-precomputed exp(rpb) table; one ACT pass does exp for both heads.
  - Softmax denominators via ones-matmul over E^T (broadcast to 64 rows);
    reciprocal_approx_fast (DVE) + one Pool multiply normalize while
    evacuating PSUM (Pool, not DVE, so stage D's DVE work stays under the
    per-iteration PE/ACT time).
  - Output projection runs transposed: out^T = Wp @ A^T (full-efficiency
    matmuls), proj bias (+ folded v_bias) added per-partition during ACT
    evacuation; host transposes the [C, M] result back. Output DMA is
    chunked per PSUM pair so the tail drains early.
  - PSUM regions are always opened with start=True (PSUM holds stale data
    from the previous tile-ring generation; there is no first-touch-write).
"""

import numpy as np
import ml_dtypes

import concourse.bass as bass
import concourse.mybir as mybir
import concourse.tile as tile
from concourse import bacc
from concourse.bass_utils import run_bass_kernel_spmd

BF16 = ml_dtypes.bfloat16
FP8 = ml_dtypes.float8_e4m3
F32 = mybir.dt.float32
BF = mybir.dt.bfloat16
F8 = mybir.dt.float8e4
DR = mybir.MatmulPerfMode.DoubleRow

B, N, C = 64, 197, 768
H, HD = 12, 64
NCORES = 8
BL = B // NCORES            # 8 batches per core
M = BL * N                  # 1576 tokens per core
MPAD = 1584
SCALE = HD ** -0.5
W8 = 64.0                   # fp8 weight prescale (power of 2; undone via ACT scale)
BCH = [(0, 128), (128, 69)]
MCHP = [(0, [(0, 512), (512, 512)]), (1024, [(0, 512), (512, 40)])]
GP_NORM = False             # Pool cannot access PSUM; normalize-multiply on DVE

_NC = None


def _build():
    nc = bacc.Bacc("TRN2", target_bir_lowering=False, debug=False)

    x8 = nc.dram_tensor("x8", [128, 3, 2, MPAD], F8, kind="ExternalInput")
    wqk8 = nc.dram_tensor("wqk8", [128, 3, 2, 2 * C], F8, kind="ExternalInput")
    x = nc.dram_tensor("x", [C, MPAD], BF, kind="ExternalInput")
    wv = nc.dram_tensor("wv", [C, C], BF, kind="ExternalInput")
    wp = nc.dram_tensor("wp", [C, C], BF, kind="ExternalInput")
    rpbe = nc.dram_tensor("rpbe", [6, 128, 2, 394], BF, kind="ExternalInput")
    qb = nc.dram_tensor("qb", [128, 6], F32, kind="ExternalInput")
    pb = nc.dram_tensor("pb", [128, 6], F32, kind="ExternalInput")
    out = nc.dram_tensor("out", [C, M], BF, kind="ExternalOutput")

    Ident = mybir.ActivationFunctionType.Identity
    Copy = mybir.ActivationFunctionType.Copy
    Exp = mybir.ActivationFunctionType.Exp

    with tile.TileContext(nc) as tc:
        with (
            tc.tile_pool(name="persist", bufs=1) as P,
            tc.tile_pool(name="et", bufs=4) as et_pool,
            tc.tile_pool(name="rcp", bufs=3) as r_pool,
            tc.tile_pool(name="ob", bufs=2) as ob_pool,
            tc.tile_pool(name="mm", bufs=2, space="PSUM") as mm,
        ):
            # ---- inputs to SBUF; stage-B operands first, col-split so the
            # ---- first matmuls can start before the full load finishes ----
            x8T = P.tile([128, 3, 2, MPAD], F8, tag="x8t", name="x8t")
            wqk_sb = P.tile([128, 3, 2, 2 * C], F8, tag="wqk", name="wqk")
            nc.sync.dma_start(x8T[:, 0:2, :, 0:1024], x8[:, 0:2, :, 0:1024])
            nc.sync.dma_start(x8T[:, 2:3, :, 0:1024], x8[:, 2:3, :, 0:1024])
            nc.sync.dma_start(wqk_sb[:, 0:2, :, 0:768], wqk8[:, 0:2, :, 0:768])
            nc.sync.dma_start(wqk_sb[:, 2:3, :, 0:768], wqk8[:, 2:3, :, 0:768])
            qb_sb = P.tile([128, 6], F32, tag="qb")
            nc.sync.dma_start(qb_sb[:, :], qb[:, :])
            nc.sync.dma_start(x8T[:, :, :, 1024:MPAD], x8[:, :, :, 1024:MPAD])
            nc.sync.dma_start(wqk_sb[:, :, :, 768 : 2 * C], wqk8[:, :, :, 768 : 2 * C])
            # remaining inputs from the (idle) gpsimd queue
            xT = [P.tile([128, MPAD], BF, tag=f"xt{t}", name=f"xt{t}") for t in range(6)]
            for t in range(6):
                nc.gpsimd.dma_start(xT[t][:, :], x[128 * t : 128 * (t + 1), :])
            wv_sb = [P.tile([128, C], BF, tag=f"wv{t}", name=f"wv{t}") for t in range(6)]
            for t in range(6):
                nc.gpsimd.dma_start(wv_sb[t][:, :], wv[128 * t : 128 * (t + 1), :])
            wp_sb = [P.tile([128, C], BF, tag=f"wp{t}", name=f"wp{t}") for t in range(6)]
            for t in range(6):
                nc.gpsimd.dma_start(wp_sb[t][:, :], wp[128 * t : 128 * (t + 1), :])
            pb_sb = P.tile([128, 6], F32, tag="pb")
            nc.gpsimd.dma_start(pb_sb[:, :], pb[:, :])
            rpb_sb = [
                P.tile([128, 2, 394], BF, tag=f"rpb{p}", name=f"rpb{p}") for p in range(6)
            ]
            for p in range(6):
                nc.gpsimd.dma_start(rpb_sb[p][:, :, :], rpbe[p, :, :, :])

            ones64 = P.tile([128, 64], BF, tag="ones64")
            nc.gpsimd.memset(ones64[:, :], 1.0)

            # ---- stage B: qkT[j, m] = (Wqk^T)^T @ x^T  (fp8 DoubleRow) ----
            qkT = [P.tile([128, M], BF, tag=f"qkt{j}", name=f"qkt{j}") for j in range(12)]
            # m-pair outer: the first pair's 12 j-groups only need the first
            # column halves of x8/wqk8, covering the second halves' DMA time
            for m0, subs in MCHP:
                for j in range(12):
                    ps = mm.tile([128, 1024], F32, tag="ps")
                    pw = 0
                    for s0, sw in subs:
                        for t3 in range(3):
                            nc.tensor.matmul(
                                ps[:, s0 : s0 + sw],
                                lhsT=wqk_sb[:, t3, :, 128 * j : 128 * (j + 1)],
                                rhs=x8T[:, t3, :, m0 + s0 : m0 + s0 + sw],
                                start=(t3 == 0),
                                stop=(t3 == 2),
                                perf_mode=DR,
                            )
                        pw = s0 + sw
                    if j < 6:  # q rows: undo fp8 prescale & apply attn scale, add bias
                        nc.scalar.activation(
                            qkT[j][:, m0 : m0 + pw],
                            ps[:, :pw],
                            Ident,
                            bias=qb_sb[:, j : j + 1],
                            scale=SCALE / W8,
                        )
                    else:  # k rows: undo fp8 prescale
                        nc.scalar.activation(
                            qkT[j][:, m0 : m0 + pw], ps[:, :pw], Copy, scale=1.0 / W8
                        )

            # ---- stage C: v[m, d] per-batch aligned chunks (bf16) ----
            v_sb = [P.tile([128, 2 * C], BF, tag=f"v{b}", name=f"v{b}") for b in range(BL)]
            for b in range(BL):
                for ch, (off, mr) in enumerate(BCH):
                    ps = mm.tile([128, 1024], F32, tag="ps")
                    for f0, fw in [(0, 512), (512, 256)]:
                        for ct in range(6):
                            nc.tensor.matmul(
                                ps[:mr, f0 : f0 + fw],
                                lhsT=xT[ct][:, b * N + off : b * N + off + mr],
                                rhs=wv_sb[ct][:, f0 : f0 + fw],
                                start=(ct == 0),
                                stop=(ct == 5),
                            )
                    nc.scalar.activation(
                        v_sb[b][:mr, ch * C : ch * C + C], ps[:mr, 0:C], Copy
                    )

            # ---- stage D: attention, two head-pairs (q = p//2) at a time ----
            # Each p's O^T + denominators land in one half of a 2-bank psOD
            # tile, so the reciprocal and the normalize-multiply each run as
            # a single strided DVE instruction covering both p's.
            ATq = [P.tile([128, 2, M], BF, tag=f"at{q}", name=f"at{q}") for q in range(3)]
            for b in range(BL):
                for q in range(3):
                    ets = []
                    for pi in range(2):
                        p = 2 * q + pi
                        psH = mm.tile([128, 2, 512], F32, tag="ps", name="psH")
                        for hj in range(2):
                            for ch, (off, mr) in enumerate(BCH):
                                nc.tensor.matmul(
                                    psH[:mr, hj, 197 * ch : 197 * ch + N],
                                    lhsT=qkT[6 + p][
                                        64 * hj : 64 * (hj + 1), b * N + off : b * N + off + mr
                                    ],
                                    rhs=qkT[p][64 * hj : 64 * (hj + 1), b * N : b * N + N],
                                    start=True,
                                    stop=True,
                                )
                        et = et_pool.tile([128, 2, 394], BF, tag="et")
                        nc.scalar.activation(et[:, :, :], psH[:, :, 0:394], Exp)
                        nc.vector.tensor_mul(et[:, :, :], et[:, :, :], rpb_sb[p][:, :, :])
                        ets.append(et)
                    psOD = mm.tile([128, 2, 512], F32, tag="od", bufs=2, name="psOD")
                    for pi in range(2):
                        p = 2 * q + pi
                        et = ets[pi]
                        for hj in range(2):
                            tp = None if hj == 0 else (0, 64)
                            for ch, (off, mr) in enumerate(BCH):
                                nc.tensor.matmul(
                                    psOD[64 * hj : 64 * (hj + 1), pi, 0:N],
                                    lhsT=v_sb[b][
                                        :mr, ch * C + (2 * p + hj) * HD : ch * C + (2 * p + hj + 1) * HD
                                    ],
                                    rhs=et[:mr, hj, 197 * ch : 197 * ch + N],
                                    start=(ch == 0),
                                    stop=(ch == 1),
                                    tile_position=tp,
                                )
                            for ch, (off, mr) in enumerate(BCH):
                                nc.tensor.matmul(
                                    psOD[64 * hj : 64 * (hj + 1), pi, 256 : 256 + N],
                                    lhsT=ones64[:mr, :],
                                    rhs=et[:mr, hj, 197 * ch : 197 * ch + N],
                                    start=(ch == 0),
                                    stop=(ch == 1),
                                    tile_position=tp,
                                )
                    rcp = r_pool.tile([128, 2, N], F32, tag="rcp")
                    nc.vector.reciprocal_approx_fast(
                        rcp[:, :, :], psOD[:, :, 256 : 256 + N]
                    )
                    nc.vector.tensor_mul(
                        ATq[q][:, :, b * N : b * N + N], psOD[:, :, 0:N], rcp[:, :, :]
                    )

            # ---- stage E: out^T[c', m] = Wp @ A^T, bias fused in evacuation ----
            for t6 in range(6):
                for m0, subs in MCHP:
                    ob = ob_pool.tile([128, 1024], BF, tag="ob")
                    ps = mm.tile([128, 1024], F32, tag="ps")
                    pw = 0
                    for s0, sw in subs:
                        for ct in range(6):
                            nc.tensor.matmul(
                                ps[:, s0 : s0 + sw],
                                lhsT=wp_sb[ct][:, 128 * t6 : 128 * (t6 + 1)],
                                rhs=ATq[ct // 2][:, ct % 2, m0 + s0 : m0 + s0 + sw],
                                start=(ct == 0),
                                stop=(ct == 5),
                            )
                        pw = s0 + sw
                    nc.scalar.activation(
                        ob[:, 0:pw],
                        ps[:, :pw],
                        Ident,
                        bias=pb_sb[:, t6 : t6 + 1],
                    )
                    nc.sync.dma_start(
                        out[128 * t6 : 128 * (t6 + 1), m0 : m0 + pw], ob[:, 0:pw]
                    )

    nc.compile()
    return nc


def _pair8(a):
    """[256k, F] -> [128, k, 2, F] fp8: channel c -> (c%128, c//256, (c%256)//128)."""
    k = a.shape[0] // 256
    return np.ascontiguousarray(
        a.reshape(k, 2, 128, a.shape[1]).transpose(2, 0, 1, 3)
    ).astype(FP8)


def _host_prep(inputs):
    x = np.asarray(inputs["x"], np.float32)
    qkv_w = np.asarray(inputs["qkv_w"], np.float32)
    q_bias = np.asarray(inputs["q_bias"], np.float32)
    v_bias = np.asarray(inputs["v_bias"], np.float32)
    rel_table = np.asarray(inputs["rel_table"], np.float32)
    proj_w = np.asarray(inputs["proj_w"], np.float32)
    proj_b = np.asarray(inputs["proj_b"], np.float32)
    rel_index = np.asarray(inputs["rel_index"], np.int64)

    wqk8_np = _pair8(qkv_w[: 2 * C].T * W8)
    wv_np = np.ascontiguousarray(qkv_w[2 * C :].T).astype(BF16)
    wp_np = np.ascontiguousarray(proj_w.T).astype(BF16)
    qb_np = np.ascontiguousarray((q_bias * SCALE).reshape(6, 128).T).astype(np.float32)
    pb_np = np.ascontiguousarray(
        (proj_b + v_bias @ proj_w.T).reshape(6, 128).T
    ).astype(np.float32)

    rpb = rel_table[rel_index]              # [N, N, H]
    rpbT = np.transpose(rpb, (2, 1, 0))     # [H, m, n]
    rpbe_np = np.ones((6, 128, 2, 394), np.float32)
    for p in range(6):
        for hj in range(2):
            h = 2 * p + hj
            rpbe_np[p, 0:128, hj, 0:N] = np.exp(rpbT[h, 0:128, :])
            rpbe_np[p, 0:69, hj, 197 : 197 + N] = np.exp(rpbT[h, 128:N, :])
    rpbe_np = rpbe_np.astype(BF16)

    consts = {
        "wqk8": wqk8_np,
        "wv": wv_np,
        "wp": wp_np,
        "rpbe": rpbe_np,
        "qb": qb_np,
        "pb": pb_np,
    }
    in_maps = []
    for i in range(NCORES):
        xi = x[BL * i : BL * (i + 1)].reshape(M, C)
        xpad = np.zeros((MPAD, C), np.float32)
        xpad[:M] = xi
        xpT = np.ascontiguousarray(xpad.T)
        in_maps.append(
            {"x8": _pair8(xpT), "x": xpT.astype(BF16), **consts}
        )
    return in_maps


def _run(inputs, trace=False):
    global _NC
    if _NC is None:
        _NC = _build()
    in_maps = _host_prep(inputs)
    res = run_bass_kernel_spmd(_NC, in_maps, core_ids=list(range(NCORES)), trace=trace)
    outs = [
        np.asarray(res.results[i]["out"]).astype(np.float32).T.reshape(BL, N, C)
        for i in range(NCORES)
    ]
    full = np.concatenate(outs, axis=0)
    return full, res


def kernel(**inputs) -> np.ndarray:
    full, _ = _run(inputs, trace=False)
    return full


# revision 26
# speedup vs baseline: 1.1745x; 1.1745x over previous
"""BEiT-style attention (B=64, N=197, C=768, H=12, rel-pos bias) on 8 TRN2 cores.

Data-parallel over batch: 8 batch items per core, no collectives.

Key layout / engine choices:
  - The qk projection runs in fp8e4 DoubleRow perf mode (two contraction
    rows per partition, 2x PE throughput), weights pre-scaled by 64 on the
    host; the 1/64 (and the 1/sqrt(hd) attention scale for q) folds back in
    via the ACT scale operand during PSUM evacuation. The v projection and
    everything downstream stay bf16: softmax weights are near-uniform here,
    so v/proj quantization error does not average away, but q/k error only
    perturbs softmax weights (~1.6e-2 total, inside the 2e-2 gate).
  - qkv computed transposed: qkT [j, m]; per-head qT/kT [hd, n] slices are
    direct row-slices. v computed un-transposed [m, d] per batch.
  - Attention runs in the S^T layout: S^T[m, n] = k q^T (keys on
    partitions). The rel-pos bias is applied multiplicatively after exp
    (exp(S+rpb) = exp(S)*exp(rpb)) by one DVE multiply against a
    host-precomputed exp(rpb) table; one ACT pass does exp for both heads.
  - Softmax denominators via ones-matmul over E^T (broadcast to 64 rows);
    reciprocal_approx_fast (DVE) + one Pool multiply normalize while
    evacuating PSUM (Pool, not DVE, so stage D's DVE work stays under the
    per-iteration PE/ACT time).
  - Output projection runs transposed: out^T = Wp @ A^T (full-efficiency
    matmuls), proj bias (+ folded v_bias) added per-partition during ACT
    evacuation; host transposes the [C, M] result back. Output DMA is
    chunked per PSUM pair so the tail drains early.
  - PSUM regions are always opened with start=True (PSUM holds stale data
    from the previous tile-ring generation; there is no first-touch-write).
"""

import numpy as np
import ml_dtypes

import concourse.bass as bass
import concourse.mybir as mybir
import concourse.tile as tile
from concourse import bacc
from concourse.bass_utils import run_bass_kernel_spmd

BF16 = ml_dtypes.bfloat16
FP8 = ml_dtypes.float8_e4m3
F32 = mybir.dt.float32
BF = mybir.dt.bfloat16
F8 = mybir.dt.float8e4
DR = mybir.MatmulPerfMode.DoubleRow

B, N, C = 64, 197, 768
H, HD = 12, 64
NCORES = 8
BL = B // NCORES            # 8 batches per core
M = BL * N                  # 1576 tokens per core
MPAD = 1584
SCALE = HD ** -0.5
W8 = 64.0                   # fp8 weight prescale (power of 2; undone via ACT scale)
BCH = [(0, 128), (128, 69)]
MCHP = [(0, [(0, 512), (512, 512)]), (1024, [(0, 512), (512, 40)])]
GP_NORM = False             # Pool cannot access PSUM; normalize-multiply on DVE

_NC = None


def _build():
    nc = bacc.Bacc("TRN2", target_bir_lowering=False, debug=False)

    x8 = nc.dram_tensor("x8", [128, 3, 2, MPAD], F8, kind="ExternalInput")
    wqk8 = nc.dram_tensor("wqk8", [128, 3, 2, 2 * C], F8, kind="ExternalInput")
    x = nc.dram_tensor("x", [C, MPAD], BF, kind="ExternalInput")
    wv = nc.dram_tensor("wv", [C, C], BF, kind="ExternalInput")
    wp = nc.dram_tensor("wp", [C, C], BF, kind="ExternalInput")
    rpbe = nc.dram_tensor("rpbe", [6, 128, 2, 394], BF, kind="ExternalInput")
    qb = nc.dram_tensor("qb", [128, 6], F32, kind="ExternalInput")
    pb = nc.dram_tensor("pb", [128, 6], F32, kind="ExternalInput")
    out = nc.dram_tensor("out", [C, M], BF, kind="ExternalOutput")

    Ident = mybir.ActivationFunctionType.Identity
    Copy = mybir.ActivationFunctionType.Copy
    Exp = mybir.ActivationFunctionType.Exp

    with tile.TileContext(nc) as tc:
        with (
            tc.tile_pool(name="persist", bufs=1) as P,
            tc.tile_pool(name="et", bufs=4) as et_pool,
            tc.tile_pool(name="rcp", bufs=3) as r_pool,
            tc.tile_pool(name="ob", bufs=2) as ob_pool,
            tc.tile_pool(name="mm", bufs=2, space="PSUM") as mm,
        ):
            # ---- inputs to SBUF; stage-B operands first, col-split so the
            # ---- first matmuls can start before the full load finishes ----
            x8T = P.tile([128, 3, 2, MPAD], F8, tag="x8t", name="x8t")
            wqk_sb = P.tile([128, 3, 2, 2 * C], F8, tag="wqk", name="wqk")
            nc.sync.dma_start(x8T[:, 0:2, :, 0:1024], x8[:, 0:2, :, 0:1024])
            nc.sync.dma_start(x8T[:, 2:3, :, 0:1024], x8[:, 2:3, :, 0:1024])
            nc.sync.dma_start(wqk_sb[:, 0:2, :, 0:768], wqk8[:, 0:2, :, 0:768])
            nc.sync.dma_start(wqk_sb[:, 2:3, :, 0:768], wqk8[:, 2:3, :, 0:768])
            qb_sb = P.tile([128, 6], F32, tag="qb")
            nc.sync.dma_start(qb_sb[:, :], qb[:, :])
            nc.sync.dma_start(x8T[:, :, :, 1024:MPAD], x8[:, :, :, 1024:MPAD])
            nc.sync.dma_start(wqk_sb[:, :, :, 768 : 2 * C], wqk8[:, :, :, 768 : 2 * C])
            # remaining inputs from the (idle) gpsimd queue
            xT = [P.tile([128, MPAD], BF, tag=f"xt{t}", name=f"xt{t}") for t in range(6)]
            for t in range(6):
                nc.gpsimd.dma_start(xT[t][:, :], x[128 * t : 128 * (t + 1), :])
            wv_sb = [P.tile([128, C], BF, tag=f"wv{t}", name=f"wv{t}") for t in range(6)]
            for t in range(6):
                nc.gpsimd.dma_start(wv_sb[t][:, :], wv[128 * t : 128 * (t + 1), :])
            wp_sb = [P.tile([128, C], BF, tag=f"wp{t}", name=f"wp{t}") for t in range(6)]
            for t in range(6):
                nc.gpsimd.dma_start(wp_sb[t][:, :], wp[128 * t : 128 * (t + 1), :])
            pb_sb = P.tile([128, 6], F32, tag="pb")
            nc.gpsimd.dma_start(pb_sb[:, :], pb[:, :])
            rpb_sb = [
                P.tile([128, 2, 394], BF, tag=f"rpb{p}", name=f"rpb{p}") for p in range(6)
            ]
            for p in range(6):
                nc.gpsimd.dma_start(rpb_sb[p][:, :, :], rpbe[p, :, :, :])

            ones64 = P.tile([128, 64], BF, tag="ones64")
            nc.gpsimd.memset(ones64[:, :], 1.0)

            # ---- stage B: qkT[j, m] = (Wqk^T)^T @ x^T  (fp8 DoubleRow) ----
            qkT = [P.tile([128, M], BF, tag=f"qkt{j}", name=f"qkt{j}") for j in range(12)]
            # m-pair outer: the first pair's 12 j-groups only need the first
            # column halves of x8/wqk8, covering the second halves' DMA time
            for m0, subs in MCHP:
                for j in range(12):
                    ps = mm.tile([128, 1024], F32, tag="ps")
                    pw = 0
                    for s0, sw in subs:
                        for t3 in range(3):
                            nc.tensor.matmul(
                                ps[:, s0 : s0 + sw],
                                lhsT=wqk_sb[:, t3, :, 128 * j : 128 * (j + 1)],
                                rhs=x8T[:, t3, :, m0 + s0 : m0 + s0 + sw],
                                start=(t3 == 0),
                                stop=(t3 == 2),
                                perf_mode=DR,
                            )
                        pw = s0 + sw
                    if j < 6:  # q rows: undo fp8 prescale & apply attn scale, add bias
                        nc.scalar.activation(
                            qkT[j][:, m0 : m0 + pw],
                            ps[:, :pw],
                            Ident,
                            bias=qb_sb[:, j : j + 1],
                            scale=SCALE / W8,
                        )
                    else:  # k rows: undo fp8 prescale
                        nc.scalar.activation(
                            qkT[j][:, m0 : m0 + pw], ps[:, :pw], Copy, scale=1.0 / W8
                        )

            # ---- stage C: v[m, d] per-batch aligned chunks (bf16) ----
            v_sb = [P.tile([128, 2 * C], BF, tag=f"v{b}", name=f"v{b}") for b in range(BL)]
            for b in range(BL):
                for ch, (off, mr) in enumerate(BCH):
                    ps = mm.tile([128, 1024], F32, tag="ps")
                    for f0, fw in [(0, 512), (512, 256)]:
                        for ct in range(6):
                            nc.tensor.matmul(
                                ps[:mr, f0 : f0 + fw],
                                lhsT=xT[ct][:, b * N + off : b * N + off + mr],
                                rhs=wv_sb[ct][:, f0 : f0 + fw],
                                start=(ct == 0),
                                stop=(ct == 5),
                            )
                    nc.scalar.activation(
                        v_sb[b][:mr, ch * C : ch * C + C], ps[:mr, 0:C], Copy
                    )

            # ---- stage D: attention, two head-pairs (q = p//2) at a time ----
            # Each p's O^T + denominators land in one half of a 2-bank psOD
            # tile, so the reciprocal and the normalize-multiply each run as
            # a single strided DVE instruction covering both p's.
            ATq = [P.tile([128, 2, M], BF, tag=f"at{q}", name=f"at{q}") for q in range(3)]
            for b in range(BL):
                for q in range(3):
                    ets = []
                    for pi in range(2):
                        p = 2 * q + pi
                        psH = mm.tile([128, 2, 512], F32, tag="ps", name="psH")
                        for hj in range(2):
                            for ch, (off, mr) in enumerate(BCH):
                                nc.tensor.matmul(
                                    psH[:mr, hj, 197 * ch : 197 * ch + N],
                                    lhsT=qkT[6 + p][
                                        64 * hj : 64 * (hj + 1), b * N + off : b * N + off + mr
                                    ],
                                    rhs=qkT[p][64 * hj : 64 * (hj + 1), b * N : b * N + N],
                                    start=True,
                                    stop=True,
                                )
                        et = et_pool.tile([128, 2, 394], BF, tag="et")
                        nc.scalar.activation(et[:, :, :], psH[:, :, 0:394], Exp)
                        nc.vector.tensor_mul(et[:, :, :], et[:, :, :], rpb_sb[p][:, :, :])
                        ets.append(et)
                    psOD = mm.tile([128, 2, 512], F32, tag="od", bufs=2, name="psOD")
                    for pi in range(2):
                        p = 2 * q + pi
                        et = ets[pi]
                        for hj in range(2):
                            tp = None if hj == 0 else (0, 64)
                            for ch, (off, mr) in enumerate(BCH):
                                nc.tensor.matmul(
                                    psOD[64 * hj : 64 * (hj + 1), pi, 0:N],
                                    lhsT=v_sb[b][
                                        :mr, ch * C + (2 * p + hj) * HD : ch * C + (2 * p + hj + 1) * HD
                                    ],
                                    rhs=et[:mr, hj, 197 * ch : 197 * ch + N],
                                    start=(ch == 0),
                                    stop=(ch == 1),
                                    tile_position=tp,
                                )
                            for ch, (off, mr) in enumerate(BCH):
                                nc.tensor.matmul(
                                    psOD[64 * hj : 64 * (hj + 1), pi, 256 : 256 + N],
                                    lhsT=ones64[:mr, :],
                                    rhs=et[:mr, hj, 197 * ch : 197 * ch + N],
                                    start=(ch == 0),
                                    stop=(ch == 1),
                                    tile_position=tp,
                                )
                    rcp = r_pool.tile([128, 2, N], F32, tag="rcp")
                    nc.vector.reciprocal_approx_fast(
                        rcp[:, :, :], psOD[:, :, 256 : 256 + N]
                    )
                    nc.vector.tensor_mul(
                        ATq[q][:, :, b * N : b * N + N], psOD[:, :, 0:N], rcp[:, :, :]
                    )

            # ---- stage E: out^T[c', m] = Wp @ A^T, bias fused in evacuation ----
            for t6 in range(6):
                for m0, subs in MCHP:
                    ob = ob_pool.tile([128, 1024], BF, tag="ob")
                    ps = mm.tile([128, 1024], F32, tag="ps")
                    pw = 0
                    for s0, sw in subs:
                        for ct in range(6):
                            nc.tensor.matmul(
                                ps[:, s0 : s0 + sw],
                                lhsT=wp_sb[ct][:, 128 * t6 : 128 * (t6 + 1)],
                                rhs=ATq[ct // 2][:, ct % 2, m0 + s0 : m0 + s0 + sw],
                                start=(ct == 0),
                                stop=(ct == 5),
                            )
                        pw = s0 + sw
                    nc.scalar.activation(
                        ob[:, 0:pw],
                        ps[:, :pw],
                        Ident,
                        bias=pb_sb[:, t6 : t6 + 1],
                    )
                    nc.sync.dma_start(
                        out[128 * t6 : 128 * (t6 + 1), m0 : m0 + pw], ob[:, 0:pw]
                    )

    nc.compile()
    return nc


def _pair8(a):
    """[256k, F] -> [128, k, 2, F] fp8: channel c -> (c%128, c//256, (c%256)//128)."""
    k = a.shape[0] // 256
    return np.ascontiguousarray(
        a.reshape(k, 2, 128, a.shape[1]).transpose(2, 0, 1, 3)
    ).astype(FP8)


def _host_prep(inputs):
    x = np.asarray(inputs["x"], np.float32)
    qkv_w = np.asarray(inputs["qkv_w"], np.float32)
    q_bias = np.asarray(inputs["q_bias"], np.float32)
    v_bias = np.asarray(inputs["v_bias"], np.float32)
    rel_table = np.asarray(inputs["rel_table"], np.float32)
    proj_w = np.asarray(inputs["proj_w"], np.float32)
    proj_b = np.asarray(inputs["proj_b"], np.float32)
    rel_index = np.asarray(inputs["rel_index"], np.int64)

    wqk8_np = _pair8(qkv_w[: 2 * C].T * W8)
    wv_np = np.ascontiguousarray(qkv_w[2 * C :].T).astype(BF16)
    wp_np = np.ascontiguousarray(proj_w.T).astype(BF16)
    qb_np = np.ascontiguousarray((q_bias * SCALE).reshape(6, 128).T).astype(np.float32)
    pb_np = np.ascontiguousarray(
        (proj_b + v_bias @ proj_w.T).reshape(6, 128).T
    ).astype(np.float32)

    rpb = rel_table[rel_index]              # [N, N, H]
    rpbT = np.transpose(rpb, (2, 1, 0))     # [H, m, n]
    rpbe_np = np.ones((6, 128, 2, 394), np.float32)
    for p in range(6):
        for hj in range(2):
            h = 2 * p + hj
            rpbe_np[p, 0:128, hj, 0:N] = np.exp(rpbT[h, 0:128, :])
            rpbe_np[p, 0:69, hj, 197 : 197 + N] = np.exp(rpbT[h, 128:N, :])
    rpbe_np = rpbe_np.astype(BF16)

    consts = {
        "wqk8": wqk8_np,
        "wv": wv_np,
        "wp": wp_np,
        "rpbe": rpbe_np,
        "qb": qb_np,
        "pb": pb_np,
    }
    in_maps = []
    for i in range(NCORES):
        xi = x[BL * i : BL * (i + 1)].reshape(M, C)
        xpad = np.zeros((MPAD, C), np.float32)
        xpad[:M] = xi
        xpT = np.ascontiguousarray(xpad.T)
        in_maps.append(
            {"x8": _pair8(xpT), "x": xpT.astype(BF16), **consts}
        )
    return in_maps


def _run(inputs, trace=False):
    global _NC
    if _NC is None:
        _NC = _build()
    in_maps = _host_prep(inputs)
    res = run_bass_kernel_spmd(_NC, in_maps, core_ids=list(range(NCORES)), trace=trace)
    outs = [
        np.asarray(res.results[i]["out"]).astype(np.float32).T.reshape(BL, N, C)
        for i in range(NCORES)
    ]
    full = np.concatenate(outs, axis=0)
    return full, res


def kernel(**inputs) -> np.ndarray:
    full, _ = _run(inputs, trace=False)
    return full
